# revision 1
# baseline (speedup 1.0000x reference)
"""Trainium2 Bass kernel for nn_InceptionTraversal (hierarchical sphere-softmax
MoE routing + per-band sigmoid routers).

Strategy
--------
Host (numpy):
  * All distances d_s = |M_s p + u_s|^2 for the 84 spheres (4 + 16 + 64, with
    portal affines composed) are linear in the 10-feature vector
    psi = [x^2, xy, xz, y^2, yz, z^2, x, y, z, 1].  Fold alpha = 1/(2T^2+eps)
    and the per-sphere constants into one matrix.
  * The 4 per-band routers (sigmoid(x_band @ W_n + b_n)) are one block-diagonal
    matmul over the 64 spectral dims; sigmoid(x) = 0.5 + 0.5*tanh(x/2), the 0.5
    folded into the weights so only tanh is needed on-device (same ACT table
    set as exp).
  * Ship Phi = [psi(9); ones(1); spectral(64)] pre-transposed [74, Ntok] so the
    device needs no transposes at all: each 128-token group is an LDWEIGHTS
    [74,128] + one matmul with N=340 producing token-major [128tok, 84+256].
Device (per core, 16384 tokens = 128 groups, superchunks of 4 groups):
  Phase A (sqrt ACT table set): matmul d-only (N=84) -> u = lam*sqrt(d+eps)
    stashed in SBUF for all 128 groups.
  Phase B (exp/tanh set): matmul full (N=340); E = exp(-d)*sqrt8 (softmax
    numerators for Z-sums), t = d + u, H = exp(-t) (numerators incl. ray decay),
    th = tanh(r); Z3/Z2 group-sums; m3e = H1*H2*H3/(8*Z2*Z3) via
    reciprocal_approx; pre = (sum_n th + 4) * m3e with fused row-sum;
    routing = pre / sum  (Z1 and all folds cancel in the normalize).
Sharding: pure data-parallel over 8 cores (tokens split 8 ways).
"""

import sys

import numpy as np

if "/opt/trn_rl_repo" not in sys.path:
    sys.path.insert(0, "/opt/trn_rl_repo")

# ---- problem constants (hardcoded per contest contract) ----
N_DOM, N_SUB, N_CON = 4, 4, 4
SPECTRAL_DIM, N_BANDS = 64, 4
BAND_SIZE = SPECTRAL_DIM // N_BANDS
TEMP, LAM, EPS = 1.0, 0.1, 1e-8
ALPHA = 1.0 / (2.0 * TEMP * TEMP + EPS)
N_CORES = 8
B, S = 16, 8192
NTOK = B * S
TPC = NTOK // N_CORES          # tokens per core = 16384
GRP = 128                      # tokens per matmul group
G = 4                          # groups per superchunk (PSUM ping-pong of 4 banks)
NSC = TPC // (GRP * G)         # superchunks = 32
NS = 84                        # spheres (4 + 16 + 64)
NR = 256                       # router logits (64 leaves x 4 bands, (k,n) order)
NCOL = NS + NR                 # matmul N = 340
KF = 74                        # Phi rows: 9 psi + 1 ones + 64 spectral
KD = 10                        # rows used by the distance matmul

_compiled = {}


def _host_matrices(centers1, centers2, centers3, portal1_T, portal2_T,
                   W_bands, b_bands, band_weights):
    """Build Wd [10,84], Wc [74,340], plus fold info. float64 internally."""
    c1 = centers1.astype(np.float64)
    c2 = centers2.astype(np.float64)
    c3 = centers3.astype(np.float64)
    A1 = portal1_T[:, :, :3].astype(np.float64)
    b1 = portal1_T[:, :, 3].astype(np.float64)
    A2 = portal2_T[:, :, :3].astype(np.float64)
    b2 = portal2_T[:, :, 3].astype(np.float64)

    Ms = np.zeros((NS, 3, 3))
    us = np.zeros((NS, 3))
    s = 0
    for j in range(N_DOM):                     # level 1
        Ms[s] = np.eye(3)
        us[s] = -c1[j]
        s += 1
    for j in range(N_DOM):                     # level 2
        for l in range(N_SUB):
            Ms[s] = A1[j]
            us[s] = b1[j] - c2[j * N_SUB + l]
            s += 1
    for j in range(N_DOM):                     # level 3
        for l in range(N_SUB):
            jl = j * N_SUB + l
            M = A2[jl] @ A1[j]
            v = A2[jl] @ b1[j] + b2[jl]
            for m in range(N_CON):
                Ms[s] = M
                us[s] = v - c3[jl * N_CON + m]
                s += 1
    assert s == NS

    # d_s(p) = psi(p) . Wd[:, s] with psi = [x2,xy,xz,y2,yz,z2,x,y,z,1]
    Wd = np.zeros((KD, NS))
    for i in range(NS):
        Q = Ms[i].T @ Ms[i]
        lin = 2.0 * (Ms[i].T @ us[i])
        Wd[:, i] = [Q[0, 0], 2 * Q[0, 1], 2 * Q[0, 2], Q[1, 1], 2 * Q[1, 2],
                    Q[2, 2], lin[0], lin[1], lin[2], us[i] @ us[i]]
    Wd *= ALPHA                                # PSUM d-cols = alpha * d_true

    w = np.exp(band_weights.astype(np.float64))
    w = w / w.sum()
    equal_w = bool(np.allclose(w, w[0], rtol=1e-6, atol=1e-9))

    Wc = np.zeros((KF, NCOL))
    Wc[0:KD, 0:NS] = Wd
    # router cols: col NS + k*4 + n = 0.5 * (x_band_n . W_bands[n,:,k] + b[n,k])
    Wr = np.zeros((SPECTRAL_DIM, SPECTRAL_DIM, N_BANDS))
    for n in range(N_BANDS):
        Wr[n * BAND_SIZE:(n + 1) * BAND_SIZE, :, n] = 0.5 * W_bands[n].astype(np.float64)
    Wc[KD:KF, NS:NCOL] = Wr.reshape(SPECTRAL_DIM, NR)
    Wc[KD - 1, NS:NCOL] = 0.5 * b_bands.astype(np.float64).T.reshape(NR)
    return (Wd.astype(np.float32), Wc.astype(np.float32), equal_w,
            w.astype(np.float32))


def _host_phi(pos_3d, spectral_color):
    """Phi [74, NTOK] f32: rows [x2,xy,xz,y2,yz,z2,x,y,z,1, spectral...]."""
    p = pos_3d.reshape(-1, 3).astype(np.float32)
    x, y, z = p[:, 0], p[:, 1], p[:, 2]
    phi = np.empty((KF, NTOK), dtype=np.float32)
    phi[0] = x * x
    phi[1] = x * y
    phi[2] = x * z
    phi[3] = y * y
    phi[4] = y * z
    phi[5] = z * z
    phi[6] = x
    phi[7] = y
    phi[8] = z
    phi[9] = 1.0
    phi[KD:] = spectral_color.reshape(-1, SPECTRAL_DIM).astype(np.float32).T
    return np.ascontiguousarray(phi)


def _build_module(equal_w, w_vec):
    import concourse.bacc as bacc
    import concourse.mybir as mybir
    import concourse.tile as tile

    f32 = mybir.dt.float32
    AF = mybir.ActivationFunctionType
    OP = mybir.AluOpType

    nc = bacc.Bacc("TRN2", target_bir_lowering=False)
    phi_d = nc.dram_tensor("phi", [KF, TPC], f32, kind="ExternalInput")
    wd_d = nc.dram_tensor("wd", [KD, NS], f32, kind="ExternalInput")
    wc_d = nc.dram_tensor("wc", [KF, NCOL], f32, kind="ExternalInput")
    out_d = nc.dram_tensor("routing", [TPC, SPECTRAL_DIM], f32, kind="ExternalOutput")

    # numeric folds
    sq_scale = (LAM * LAM) / ALPHA          # u = sqrt(sq_scale*dps + sq_bias)
    sq_bias = LAM * LAM * EPS
    cfold = 8.0 if equal_w else 2.0         # E' = sqrt(cfold)*E so R = 1/(cfold*Z2*Z3)
    e_bias = 0.5 * float(np.log(cfold))
    pre_add = 4.0 if equal_w else 1.0       # refr_true = (1/cfold)*(S + pre_add)

    CH = G * GRP                            # 512 tokens per superchunk

    # activation() turns float biases into const APs — register ours.
    for cval in (sq_bias, e_bias):
        if (f32, cval) not in nc.const_aps.aps:
            ct = nc.alloc_sbuf_tensor(f"const-f32-{cval}", [128, 1], f32)
            nc.gpsimd.memset(ct.ap(), cval)
            nc.const_aps.aps[(f32, cval)] = ct.ap()
    nc.all_engine_barrier()

    with tile.TileContext(nc) as tc:
        with (
            tc.tile_pool(name="const", bufs=1) as constp,
            tc.tile_pool(name="stash", bufs=1) as stashp,
            tc.tile_pool(name="io", bufs=3) as iop,
            tc.tile_pool(name="work", bufs=3) as wp,
            tc.tile_pool(name="ps", bufs=2, space="PSUM") as psp,
        ):
            wd_sb = constp.tile([KD, NS], f32)
            nc.sync.dma_start(wd_sb[:], wd_d[:])
            wc_sb = constp.tile([KF, NCOL], f32)
            nc.sync.dma_start(wc_sb[:], wc_d[:])
            if not equal_w:
                wt_sb = constp.tile([GRP, N_BANDS], f32)
                wt_dram = nc.dram_tensor("wt", [1, N_BANDS], f32, kind="ExternalInput")
                nc.sync.dma_start(wt_sb[:], wt_dram[:].partition_broadcast(GRP))

            u_stash = stashp.tile([GRP, TPC // GRP * NS], f32)  # 84 f32 per group

            # ---------------- Phase A: sqrt table set ----------------
            for sc in range(NSC):
                phiA = iop.tile([KD, CH], f32, tag="phiA")
                nc.sync.dma_start(phiA[:], phi_d[0:KD, sc * CH:(sc + 1) * CH])
                psA = psp.tile([GRP, G, 512], f32, tag="ps")
                for g in range(G):
                    nc.tensor.matmul(
                        psA[:, g, 0:NS],
                        phiA[:, g * GRP:(g + 1) * GRP],
                        wd_sb[:],
                        start=True, stop=True,
                    )
                ust = u_stash[:, sc * (G * NS):(sc + 1) * (G * NS)]
                nc.scalar.activation(
                    ust.rearrange("p (g c) -> p g c", g=G),
                    psA[:, :, 0:NS],
                    AF.Sqrt, bias=sq_bias, scale=sq_scale,
                )

            tc.strict_bb_all_engine_barrier()

            # ---------------- Phase B: exp/tanh table set ----------------
            for sc in range(NSC):
                phiB = iop.tile([KF, CH], f32, tag="phiB")
                nc.sync.dma_start(phiB[:], phi_d[:, sc * CH:(sc + 1) * CH])
                psB = psp.tile([GRP, G, 512], f32, tag="ps")
                for g in range(G):
                    nc.tensor.matmul(
                        psB[:, g, 0:NCOL],
                        phiB[:, g * GRP:(g + 1) * GRP],
                        wc_sb[:],
                        start=True, stop=True,
                    )
                dps = psB[:, :, 0:NS]
                rps = psB[:, :, NS:NCOL]

                E = wp.tile([GRP, G, NS - 4], f32, tag="E")
                nc.scalar.activation(E[:], psB[:, :, 4:NS], AF.Exp,
                                     bias=e_bias, scale=-1.0)

                t = wp.tile([GRP, G, NS], f32, tag="t")
                ust = u_stash[:, sc * (G * NS):(sc + 1) * (G * NS)]
                nc.vector.tensor_tensor(
                    t[:], dps, ust.rearrange("p (g c) -> p g c", g=G), OP.add)

                H = wp.tile([GRP, G, NS], f32, tag="H")
                nc.scalar.activation(H[:], t[:], AF.Exp, scale=-1.0)

                th = wp.tile([GRP, G, NR], f32, tag="th")
                nc.scalar.activation(th[:], rps, AF.Tanh)

                E3v = E[:, :, 16:80].rearrange("p g (jl m) -> p g jl m", m=4)
                z01 = wp.tile([GRP, G, 16], f32, tag="z01")
                nc.gpsimd.tensor_tensor(z01[:], E3v[:, :, :, 0], E3v[:, :, :, 1], OP.add)
                z23 = wp.tile([GRP, G, 16], f32, tag="z23")
                nc.gpsimd.tensor_tensor(z23[:], E3v[:, :, :, 2], E3v[:, :, :, 3], OP.add)
                Z3 = wp.tile([GRP, G, 16], f32, tag="Z3")
                nc.gpsimd.tensor_tensor(Z3[:], z01[:], z23[:], OP.add)
                Z2 = wp.tile([GRP, G, 4], f32, tag="Z2")
                nc.vector.tensor_reduce(
                    Z2[:], E[:, :, 0:16].rearrange("p g (j l) -> p g j l", l=4),
                    mybir.AxisListType.X, OP.add)

                D = wp.tile([GRP, G, 16], f32, tag="D")
                nc.vector.tensor_tensor(
                    D.rearrange("p g (j l) -> p g j l", l=4),
                    Z3.rearrange("p g (j l) -> p g j l", l=4),
                    Z2[:].unsqueeze(3).broadcast_to((GRP, G, 4, 4)),
                    OP.mult)
                R = wp.tile([GRP, G, 16], f32, tag="R")
                nc.vector.reciprocal_approx_fast(R[:], D[:])

                a = wp.tile([GRP, G, 16], f32, tag="a")
                nc.vector.tensor_tensor(a[:], H[:, :, 4:20], R[:], OP.mult)
                a2 = wp.tile([GRP, G, 16], f32, tag="a2")
                nc.vector.tensor_tensor(
                    a2.rearrange("p g (j l) -> p g j l", l=4),
                    a.rearrange("p g (j l) -> p g j l", l=4),
                    H[:, :, 0:4].unsqueeze(3).broadcast_to((GRP, G, 4, 4)),
                    OP.mult)
                m3e = wp.tile([GRP, G, 64], f32, tag="m3e")
                nc.vector.tensor_tensor(
                    m3e.rearrange("p g (jl m) -> p g jl m", m=4),
                    H[:, :, 20:NS].rearrange("p g (jl m) -> p g jl m", m=4),
                    a2[:].unsqueeze(3).broadcast_to((GRP, G, 16, 4)),
                    OP.mult)

                sth = wp.tile([GRP, G, 64], f32, tag="sth")
                if equal_w:
                    # band-sum tree on GPSIMD (keeps DVE free); th is SBUF-only
                    thv = th[:].rearrange("p g (k n) -> p g k n", n=4)
                    s01 = wp.tile([GRP, G, 64], f32, tag="s01")
                    nc.gpsimd.tensor_tensor(s01[:], thv[:, :, :, 0], thv[:, :, :, 1], OP.add)
                    s23 = wp.tile([GRP, G, 64], f32, tag="s23")
                    nc.gpsimd.tensor_tensor(s23[:], thv[:, :, :, 2], thv[:, :, :, 3], OP.add)
                    nc.gpsimd.tensor_tensor(sth[:], s01[:], s23[:], OP.add)
                else:
                    thw = wp.tile([GRP, G, NR], f32, tag="thw")
                    nc.vector.tensor_tensor(
                        thw.rearrange("p g (k n) -> p g k n", n=4),
                        th[:].rearrange("p g (k n) -> p g k n", n=4),
                        wt_sb[:].unsqueeze(1).unsqueeze(1).broadcast_to(
                            (GRP, G, 64, N_BANDS)),
                        OP.mult)
                    nc.vector.tensor_reduce(
                        sth[:], thw.rearrange("p g (k n) -> p g k n", n=4),
                        mybir.AxisListType.X, OP.add)

                pre = wp.tile([GRP, G, 64], f32, tag="pre")
                ssum = wp.tile([GRP, G], f32, tag="ssum")
                for g in range(G):
                    nc.vector.scalar_tensor_tensor(
                        pre[:, g, :], sth[:, g, :], pre_add, m3e[:, g, :],
                        OP.add, OP.mult, accum_out=ssum[:, g:g + 1])
                rcp = wp.tile([GRP, G], f32, tag="rcp")
                nc.vector.reciprocal_approx_fast(rcp[:], ssum[:])

                ot = wp.tile([GRP, G, 64], f32, tag="ot")
                for g in range(G):
                    nc.gpsimd.tensor_scalar_mul(
                        ot[:, g, :], pre[:, g, :], rcp[:, g:g + 1])

                nc.sync.dma_start(
                    out_d[sc * CH:(sc + 1) * CH, :].rearrange(
                        "(g p) k -> p g k", p=GRP),
                    ot[:])

    nc.finalize()
    return nc


def _get_compiled(equal_w, w_vec):
    key = (equal_w, tuple(np.round(w_vec.astype(np.float64), 9)))
    if key not in _compiled:
        _compiled[key] = _build_module(equal_w, w_vec)
    return _compiled[key]


def kernel(pos_3d, spectral_color, centers1, centers2, centers3,
           portal1_T, portal2_T, W_bands, b_bands, band_weights):
    from concourse.bass_utils import run_bass_kernel_spmd

    pos_3d = np.asarray(pos_3d)
    spectral_color = np.asarray(spectral_color)
    Wd, Wc, equal_w, w_vec = _host_matrices(
        np.asarray(centers1), np.asarray(centers2), np.asarray(centers3),
        np.asarray(portal1_T), np.asarray(portal2_T),
        np.asarray(W_bands), np.asarray(b_bands), np.asarray(band_weights))
    phi = _host_phi(pos_3d, spectral_color)

    nc = _get_compiled(equal_w, w_vec)

    in_maps = []
    for c in range(N_CORES):
        m = {
            "phi": np.ascontiguousarray(phi[:, c * TPC:(c + 1) * TPC]),
            "wd": Wd,
            "wc": Wc,
        }
        if not equal_w:
            m["wt"] = w_vec.reshape(1, N_BANDS)
        in_maps.append(m)

    res = run_bass_kernel_spmd(nc, in_maps, core_ids=list(range(N_CORES)))
    outs = [res.results[c]["routing"] for c in range(N_CORES)]
    full = np.concatenate(outs, axis=0).reshape(B, S, SPECTRAL_DIM)
    return full.astype(np.float32)


if __name__ == "__main__":
    rng = np.random.default_rng(0)
    sys.path.insert(0, "/root/problem")
    import reference
    inputs = {k: np.asarray(v) for k, v in reference.setup_inputs().items()}
    out = kernel(**inputs)
    exp = np.asarray(reference.reference(**inputs))
    err = np.max(np.abs(out - exp)) / max(np.max(np.abs(exp)), 1e-12)
    print("Relative error:", err)



# revision 2
# speedup vs baseline: 2.2167x; 2.2167x over previous
"""Trainium2 Bass kernel for nn_InceptionTraversal (hierarchical sphere-softmax
MoE routing + per-band sigmoid routers).

Strategy (v2 — bf16 single-pass-per-phase redesign)
---------------------------------------------------
Math: routing_k  ∝  exp(-T3_k) * R_jl(k) * (4 + sth_k), normalized over k.
  T3_k = alpha*(d1+d2+d3) + lam*(sqrt(d1)+sqrt(d2)+sqrt(d3))  (path sums)
  R_jl = 1/(Z2_j * Z3_jl),  Z = per-parent softmax denominators
  sth_k = sum_n tanh(r_nk/2)     (sigmoid routers, 0.5 folds cancel)
Z1, the 1/8 refr scale and all constant folds cancel in the final normalize.

Device plan (per core, 16384 tokens, 128-token groups, 4-group superchunks):
  * All matmuls bf16 (4x faster than fp32 LOW/HIGH on the PE, FWL weight
    loads).  Distance precision is restored with a hi/lo split: psi rows are
    shipped as [psi_hi; psi_hi; psi_lo] and the distance weight rows as
    [Wd_hi; Wd_lo; Wd_hi], so x = psi@Wd is exact to ~bf16^2 (validated
    5e-3 end-to-end vs 2e-2 tolerance).
  * Phase A (sqrt ACT table): matmul K=30 N=84 -> x for all 84 spheres;
    ACT sqrt -> u = lam*sqrt(d+eps) (bf16); DVE path-sums U3 = u1+u2+u3
    per leaf -> bf16 stash (DVE is otherwise idle in this phase).
  * Phase B (exp/tanh table): matmul K=94 N=400:
      cols [0:80)    x_d for spheres 4..84 (E = exp(-x) -> Z2/Z3)
      cols [80:144)  xS = alpha*(d1+d2+d3) per leaf (path-sum weights)
      cols [144:400) r router logits, band-major (n,k)
    ACT: E, th = tanh(r), H = exp(-(xS+U3)); DVE: T3 add, Z-reduces,
    reciprocal, band tree, (4+sth)*m3e with fused row-sum, normalize.
  * All intermediates bf16 (except the reciprocal path, fp32), output bf16
    upcast on host.
Sharding: pure data-parallel over 8 cores (tokens split 8 ways).
"""

import sys

import numpy as np

if "/opt/trn_rl_repo" not in sys.path:
    sys.path.insert(0, "/opt/trn_rl_repo")

# ---- problem constants (hardcoded per contest contract) ----
N_DOM, N_SUB, N_CON = 4, 4, 4
SPECTRAL_DIM, N_BANDS = 64, 4
BAND_SIZE = SPECTRAL_DIM // N_BANDS
TEMP, LAM, EPS = 1.0, 0.1, 1e-8
ALPHA = 1.0 / (2.0 * TEMP * TEMP + EPS)
N_CORES = 8
B, S = 16, 8192
NTOK = B * S
TPC = NTOK // N_CORES          # tokens per core = 16384
GRP = 128                      # tokens per matmul group
G = 4                          # groups per superchunk (PSUM ping-pong)
NSC = TPC // (GRP * G)         # superchunks = 32
NS = 84                        # spheres (4 + 16 + 64)
NLEAF = 64
NR = 256                       # router logits, band-major (n,k)
KD = 10                        # psi features [x2,xy,xz,y2,yz,z2,x,y,z,1]
KA = 3 * KD                    # phase-A rows: [psi_hi; psi_hi; psi_lo]
KB = KA + SPECTRAL_DIM         # phase-B rows: + spectral (bf16)
NE = 80                        # E cols (spheres 4..84)
NB = NE + NLEAF + NR           # phase-B matmul N = 400

_compiled = {}


def _bf16(x):
    x = np.asarray(x, np.float32)
    i = x.view(np.uint32)
    r = ((i >> 16) + ((i >> 15) & 1)).astype(np.uint32) << 16
    return r.view(np.float32)


def _host_matrices(centers1, centers2, centers3, portal1_T, portal2_T,
                   W_bands, b_bands, band_weights):
    """Build WA [30,84] (phase A), WB [94,400] (phase B), both bf16-valued
    fp32 arrays, plus the band-weight info."""
    c1 = centers1.astype(np.float64)
    c2 = centers2.astype(np.float64)
    c3 = centers3.astype(np.float64)
    A1 = portal1_T[:, :, :3].astype(np.float64)
    b1 = portal1_T[:, :, 3].astype(np.float64)
    A2 = portal2_T[:, :, :3].astype(np.float64)
    b2 = portal2_T[:, :, 3].astype(np.float64)

    Ms = np.zeros((NS, 3, 3))
    us = np.zeros((NS, 3))
    s = 0
    for j in range(N_DOM):                     # level 1
        Ms[s] = np.eye(3)
        us[s] = -c1[j]
        s += 1
    for j in range(N_DOM):                     # level 2
        for l in range(N_SUB):
            Ms[s] = A1[j]
            us[s] = b1[j] - c2[j * N_SUB + l]
            s += 1
    for j in range(N_DOM):                     # level 3
        for l in range(N_SUB):
            jl = j * N_SUB + l
            M = A2[jl] @ A1[j]
            v = A2[jl] @ b1[j] + b2[jl]
            for m in range(N_CON):
                Ms[s] = M
                us[s] = v - c3[jl * N_CON + m]
                s += 1
    assert s == NS

    # x_s(p) = psi(p) . Wd[:, s],  psi = [x2,xy,xz,y2,yz,z2,x,y,z,1]
    Wd = np.zeros((KD, NS))
    for i in range(NS):
        Q = Ms[i].T @ Ms[i]
        lin = 2.0 * (Ms[i].T @ us[i])
        Wd[:, i] = [Q[0, 0], 2 * Q[0, 1], 2 * Q[0, 2], Q[1, 1], 2 * Q[1, 2],
                    Q[2, 2], lin[0], lin[1], lin[2], us[i] @ us[i]]
    Wd *= ALPHA                                # PSUM x = alpha * d_true

    # per-leaf path sums: WdS[:, k] = Wd1[j] + Wd2[jl] + Wd3[jlm]
    WdS = np.zeros((KD, NLEAF))
    for j in range(N_DOM):
        for l in range(N_SUB):
            jl = j * N_SUB + l
            for m in range(N_CON):
                k = jl * N_CON + m
                WdS[:, k] = Wd[:, j] + Wd[:, 4 + jl] + Wd[:, 20 + k]

    def hl3(W):  # hi/lo 3-block for [psi_hi; psi_hi; psi_lo] rows
        hi = _bf16(W)
        lo = _bf16(W - hi)
        return np.concatenate([hi, lo, hi], axis=0)

    WA = np.zeros((KA, NS), np.float32)
    WA[:] = hl3(Wd)

    WB = np.zeros((KB, NB), np.float32)
    WB[0:KA, 0:NE] = hl3(Wd[:, 4:NS])
    WB[0:KA, NE:NE + NLEAF] = hl3(WdS)
    # router cols, band-major: col NE+64 + n*64 + k = 0.5*(x_n.W[n,:,k] + b[n,k])
    Wr = np.zeros((SPECTRAL_DIM, NR))
    for n in range(N_BANDS):
        Wr[n * BAND_SIZE:(n + 1) * BAND_SIZE, n * NLEAF:(n + 1) * NLEAF] = \
            0.5 * W_bands[n].astype(np.float64)
    WB[KA:KB, NE + NLEAF:NB] = _bf16(Wr)
    # bias via the psi const row (psi_hi row 9 == 1.0)
    WB[KD - 1, NE + NLEAF:NB] = _bf16(
        0.5 * b_bands.astype(np.float64).reshape(NR))

    w = np.exp(band_weights.astype(np.float64))
    w = w / w.sum()
    equal_w = bool(np.allclose(w, w[0], rtol=1e-6, atol=1e-9))
    return WA, WB, equal_w, w.astype(np.float32)


def _host_phi(pos_3d, spectral_color):
    """phi [94, NTOK] f32 (bf16-valued): [psi_hi(10); psi_hi(10); psi_lo(10);
    spectral(64)]."""
    p = pos_3d.reshape(-1, 3).astype(np.float32)
    x, y, z = p[:, 0], p[:, 1], p[:, 2]
    psi = np.empty((KD, NTOK), dtype=np.float32)
    psi[0] = x * x
    psi[1] = x * y
    psi[2] = x * z
    psi[3] = y * y
    psi[4] = y * z
    psi[5] = z * z
    psi[6] = x
    psi[7] = y
    psi[8] = z
    psi[9] = 1.0
    hi = _bf16(psi)
    lo = _bf16(psi - hi)
    phi = np.empty((KB, NTOK), dtype=np.float32)
    phi[0:KD] = hi
    phi[KD:2 * KD] = hi
    phi[2 * KD:KA] = lo
    phi[KA:KB] = _bf16(spectral_color.reshape(-1, SPECTRAL_DIM).T)
    return np.ascontiguousarray(phi)


def _build_module(equal_w, w_vec):
    import concourse.bacc as bacc
    import concourse.mybir as mybir
    import concourse.tile as tile

    f32 = mybir.dt.float32
    bf = mybir.dt.bfloat16
    AF = mybir.ActivationFunctionType
    OP = mybir.AluOpType

    nc = bacc.Bacc("TRN2", target_bir_lowering=False)
    phi_d = nc.dram_tensor("phi", [KB, TPC], bf, kind="ExternalInput")
    wa_d = nc.dram_tensor("wa", [KA, NS], bf, kind="ExternalInput")
    wb_d = nc.dram_tensor("wb", [KB, NB], bf, kind="ExternalInput")
    out_d = nc.dram_tensor("routing", [TPC, NLEAF], bf, kind="ExternalOutput")

    sq_scale = (LAM * LAM) / ALPHA          # u = sqrt(sq_scale*x + sq_bias)
    sq_bias = LAM * LAM * EPS
    CH = G * GRP                            # 512 tokens per superchunk

    # activation() turns float biases into const APs — register ours.
    for cval in (sq_bias,):
        if (f32, cval) not in nc.const_aps.aps:
            ct = nc.alloc_sbuf_tensor(f"const-f32-{cval}", [128, 1], f32)
            nc.gpsimd.memset(ct.ap(), cval)
            nc.const_aps.aps[(f32, cval)] = ct.ap()
    nc.all_engine_barrier()

    with tile.TileContext(nc) as tc:
        with (
            tc.tile_pool(name="const", bufs=1) as constp,
            tc.tile_pool(name="stash", bufs=1) as stashp,
            tc.tile_pool(name="io", bufs=3) as iop,
            tc.tile_pool(name="work", bufs=3) as wp,
            tc.tile_pool(name="ps", bufs=2, space="PSUM") as psp,
        ):
            wa_sb = constp.tile([KA, NS], bf)
            nc.sync.dma_start(wa_sb[:], wa_d[:])
            wb_sb = constp.tile([KB, NB], bf)
            nc.sync.dma_start(wb_sb[:], wb_d[:])

            u3_stash = stashp.tile([GRP, NSC * G * NLEAF], bf)

            # ---------------- Phase A: sqrt table set ----------------
            for sc in range(NSC):
                phiA = iop.tile([KA, CH], bf, tag="phiA")
                nc.sync.dma_start(phiA[:], phi_d[0:KA, sc * CH:(sc + 1) * CH])
                psA = psp.tile([GRP, G, 512], f32, tag="ps")
                for g in range(G):
                    nc.tensor.matmul(
                        psA[:, g, 0:NS],
                        phiA[:, g * GRP:(g + 1) * GRP],
                        wa_sb[:],
                        start=True, stop=True,
                    )
                u = wp.tile([GRP, G, NS], bf, tag="u")
                nc.scalar.activation(u[:], psA[:, :, 0:NS],
                                     AF.Sqrt, bias=sq_bias, scale=sq_scale)
                U2 = wp.tile([GRP, G, 16], bf, tag="U2")
                nc.vector.tensor_tensor(
                    U2.rearrange("p g (j l) -> p g j l", l=4),
                    u[:, :, 4:20].rearrange("p g (j l) -> p g j l", l=4),
                    u[:, :, 0:4].unsqueeze(3).broadcast_to((GRP, G, 4, 4)),
                    OP.add)
                ust = u3_stash[:, sc * (G * NLEAF):(sc + 1) * (G * NLEAF)]
                nc.vector.tensor_tensor(
                    ust.rearrange("p (g jl m) -> p g jl m", g=G, m=4),
                    u[:, :, 20:NS].rearrange("p g (jl m) -> p g jl m", m=4),
                    U2[:].unsqueeze(3).broadcast_to((GRP, G, 16, 4)),
                    OP.add)

            tc.strict_bb_all_engine_barrier()

            # ---------------- Phase B: exp/tanh table set ----------------
            for sc in range(NSC):
                phiB = iop.tile([KB, CH], bf, tag="phiB")
                nc.sync.dma_start(phiB[:], phi_d[:, sc * CH:(sc + 1) * CH])
                psB = psp.tile([GRP, G, 512], f32, tag="ps")
                for g in range(G):
                    nc.tensor.matmul(
                        psB[:, g, 0:NB],
                        phiB[:, g * GRP:(g + 1) * GRP],
                        wb_sb[:],
                        start=True, stop=True,
                    )

                E = wp.tile([GRP, G, NE], bf, tag="E")
                nc.scalar.activation(E[:], psB[:, :, 0:NE], AF.Exp, scale=-1.0)

                th = wp.tile([GRP, G, NR], bf, tag="th")
                nc.scalar.activation(th[:], psB[:, :, NE + NLEAF:NB], AF.Tanh)

                ust = u3_stash[:, sc * (G * NLEAF):(sc + 1) * (G * NLEAF)]
                T3 = wp.tile([GRP, G, NLEAF], f32, tag="T3")
                nc.vector.tensor_tensor(
                    T3[:], psB[:, :, NE:NE + NLEAF],
                    ust.rearrange("p (g k) -> p g k", g=G), OP.add)
                H = wp.tile([GRP, G, NLEAF], bf, tag="H")
                nc.scalar.activation(H[:], T3[:], AF.Exp, scale=-1.0)

                Z3 = wp.tile([GRP, G, 16], f32, tag="Z3")
                nc.vector.tensor_reduce(
                    Z3[:], E[:, :, 16:NE].rearrange("p g (jl m) -> p g jl m", m=4),
                    mybir.AxisListType.X, OP.add)
                Z2 = wp.tile([GRP, G, 4], f32, tag="Z2")
                nc.vector.tensor_reduce(
                    Z2[:], E[:, :, 0:16].rearrange("p g (j l) -> p g j l", l=4),
                    mybir.AxisListType.X, OP.add)
                D = wp.tile([GRP, G, 16], f32, tag="D")
                nc.vector.tensor_tensor(
                    D.rearrange("p g (j l) -> p g j l", l=4),
                    Z3.rearrange("p g (j l) -> p g j l", l=4),
                    Z2[:].unsqueeze(3).broadcast_to((GRP, G, 4, 4)),
                    OP.mult)
                R = wp.tile([GRP, G, 16], f32, tag="R")
                nc.vector.reciprocal_approx_fast(R[:], D[:])
                Rb = wp.tile([GRP, G, 16], bf, tag="Rb")
                nc.vector.tensor_scalar_mul(Rb[:], R[:], 1.0)

                m3e = wp.tile([GRP, G, NLEAF], bf, tag="m3e")
                nc.vector.tensor_tensor(
                    m3e.rearrange("p g (jl m) -> p g jl m", m=4),
                    H[:].rearrange("p g (jl m) -> p g jl m", m=4),
                    Rb[:].unsqueeze(3).broadcast_to((GRP, G, 16, 4)),
                    OP.mult)

                # band tree: th is band-major [n=4, k=64]
                if equal_w:
                    thw = th
                else:
                    thw = wp.tile([GRP, G, NR], bf, tag="thw")
                    for n in range(N_BANDS):
                        nc.vector.tensor_scalar_mul(
                            thw[:, :, n * NLEAF:(n + 1) * NLEAF],
                            th[:, :, n * NLEAF:(n + 1) * NLEAF],
                            float(w_vec[n] * N_BANDS))
                z01 = wp.tile([GRP, G, NLEAF], bf, tag="z01")
                nc.vector.tensor_tensor(
                    z01[:], thw[:, :, 0:64], thw[:, :, 64:128], OP.add)
                z23 = wp.tile([GRP, G, NLEAF], bf, tag="z23")
                nc.vector.tensor_tensor(
                    z23[:], thw[:, :, 128:192], thw[:, :, 192:256], OP.add)
                sth = wp.tile([GRP, G, NLEAF], bf, tag="sth")
                nc.vector.tensor_tensor(sth[:], z01[:], z23[:], OP.add)

                pre = wp.tile([GRP, G, NLEAF], bf, tag="pre")
                ssum = wp.tile([GRP, G], f32, tag="ssum")
                for g in range(G):
                    nc.vector.scalar_tensor_tensor(
                        pre[:, g, :], sth[:, g, :], 4.0, m3e[:, g, :],
                        OP.add, OP.mult, accum_out=ssum[:, g:g + 1])
                rcp = wp.tile([GRP, G], f32, tag="rcp")
                nc.vector.reciprocal_approx_fast(rcp[:], ssum[:])
                rcpb = wp.tile([GRP, G], bf, tag="rcpb")
                nc.vector.tensor_scalar_mul(rcpb[:], rcp[:], 1.0)

                ot = wp.tile([GRP, G, NLEAF], bf, tag="ot")
                nc.vector.tensor_tensor(
                    ot[:], pre[:],
                    rcpb[:].unsqueeze(2).broadcast_to((GRP, G, NLEAF)),
                    OP.mult)

                nc.sync.dma_start(
                    out_d[sc * CH:(sc + 1) * CH, :].rearrange(
                        "(g p) k -> p g k", p=GRP),
                    ot[:])

    nc.finalize()
    return nc


def _get_compiled(equal_w, w_vec):
    key = (equal_w, tuple(np.round(w_vec.astype(np.float64), 9)))
    if key not in _compiled:
        _compiled[key] = _build_module(equal_w, w_vec)
    return _compiled[key]


def _make_in_maps(pos_3d, spectral_color, centers1, centers2, centers3,
                  portal1_T, portal2_T, W_bands, b_bands, band_weights):
    import ml_dtypes
    WA, WB, equal_w, w_vec = _host_matrices(
        np.asarray(centers1), np.asarray(centers2), np.asarray(centers3),
        np.asarray(portal1_T), np.asarray(portal2_T),
        np.asarray(W_bands), np.asarray(b_bands), np.asarray(band_weights))
    phi = _host_phi(np.asarray(pos_3d), np.asarray(spectral_color))
    bfd = ml_dtypes.bfloat16
    WAb = WA.astype(bfd)
    WBb = WB.astype(bfd)
    phib = phi.astype(bfd)
    in_maps = []
    for c in range(N_CORES):
        in_maps.append({
            "phi": np.ascontiguousarray(phib[:, c * TPC:(c + 1) * TPC]),
            "wa": WAb,
            "wb": WBb,
        })
    return in_maps, equal_w, w_vec


def kernel(pos_3d, spectral_color, centers1, centers2, centers3,
           portal1_T, portal2_T, W_bands, b_bands, band_weights):
    from concourse.bass_utils import run_bass_kernel_spmd

    in_maps, equal_w, w_vec = _make_in_maps(
        pos_3d, spectral_color, centers1, centers2, centers3,
        portal1_T, portal2_T, W_bands, b_bands, band_weights)
    nc = _get_compiled(equal_w, w_vec)
    res = run_bass_kernel_spmd(nc, in_maps, core_ids=list(range(N_CORES)))
    outs = [np.asarray(res.results[c]["routing"], dtype=np.float32)
            for c in range(N_CORES)]
    full = np.concatenate(outs, axis=0).reshape(B, S, SPECTRAL_DIM)
    return full.astype(np.float32)


if __name__ == "__main__":
    sys.path.insert(0, "/root/problem")
    import reference
    inputs = {k: np.asarray(v) for k, v in reference.setup_inputs().items()}
    out = kernel(**inputs)
    exp = np.asarray(reference.reference(**inputs))
    err = np.max(np.abs(out - exp)) / max(np.max(np.abs(exp)), 1e-12)
    print("Relative error:", err)
